# revision 17
# baseline (speedup 1.0000x reference)
"""Trainium2 Bass kernel for nn_DualModalityEnhanced (3-branch dual-stream
cross-attention transformer, B=4 S=512 D=1024 H=16 HID=4096 NL=2).

Sharding: 12 independent (branch, batch) units, each a 2-layer dual-stream
chain. SPMD over 8 cores, 2 units per core (cores 6,7 duplicate work of
cores 0,1; host discards duplicates). All matmuls run as fp32r (TF32-like)
at full PE rate.

Self-contained: hardcodes shapes; builds + compiles the Bass program on
first call and runs it via run_bass_kernel_spmd.
"""

import numpy as np

B, S, D, H, HID, NL = 4, 512, 1024, 16, 4096, 2
HD = D // H            # 64
NBRANCH = 3
LN_EPS = 1e-5
SCALE = 1.0 / 8.0      # 1/sqrt(HD)

P = 128
TT = S // P            # 4 token tiles
DT = D // P            # 8 feature row-tiles
HT = HID // P          # 32 hidden row-tiles
NQ = 4                 # quarter splits of D columns (256 each)

_cache = {}


def _build_program():
    import contextlib
    import concourse.bass as bass
    import concourse.mybir as mybir
    import concourse.tile as tile
    from concourse import bacc
    from concourse.masks import make_identity

    f32 = mybir.dt.float32
    f32r = mybir.dt.float32r
    bf16 = mybir.dt.bfloat16
    AF = mybir.ActivationFunctionType
    ALU = mybir.AluOpType

    nc = bacc.Bacc("TRN2", target_bir_lowering=False, debug=False, num_devices=8)

    # ---------------- external I/O ----------------
    m_in, outs, scr = {}, {}, {}
    for u in ("a", "b"):
        for s in (1, 2):
            m_in[(u, s)] = nc.dram_tensor(f"m{s}{u}", [S, D], f32, kind="ExternalInput")
            outs[(u, s)] = nc.dram_tensor(f"o{s}{u}", [S, D], f32, kind="ExternalOutput")
            scr[(u, s)] = nc.dram_tensor(f"scr{s}{u}", [S, D], f32)

    Wd = {}
    for n in ("WQ", "WK", "WV1", "WV2", "WO1", "WO2"):
        Wd[n] = nc.dram_tensor(n, [NL, D, D], bf16, kind="ExternalInput")
    # host-pretiled FFN weights (bf16, LN gamma folded into W1):
    #   FsW1t[l, ht] = FsW1[l][:, ht*128:(ht+1)*128]   -> [NL, HT, D, P]
    #   FsW2t[l, dm] = FsW2[l][:, dm*128:(dm+1)*128]   -> [NL, DT, HID, P]
    for n in ("F1W1t", "F2W1t"):
        Wd[n] = nc.dram_tensor(n, [NL, HT, D, P], bf16, kind="ExternalInput")
    for n in ("F1W2t", "F2W2t"):
        Wd[n] = nc.dram_tensor(n, [NL, DT, HID, P], bf16, kind="ExternalInput")
    for n in ("bQ", "bK", "bV1", "bV2", "bO1", "bO2", "F1b2", "F2b2"):
        Wd[n] = nc.dram_tensor(n, [NL, D], f32, kind="ExternalInput")
    for n in ("F1b1", "F2b1"):
        Wd[n] = nc.dram_tensor(n, [NL, HID], f32, kind="ExternalInput")

    with tile.TileContext(nc) as tc, contextlib.ExitStack() as ctx:
        A = ctx.enter_context(tc.tile_pool(name="arena", bufs=7))
        E = ctx.enter_context(tc.tile_pool(name="eqk", bufs=5))
        WP = ctx.enter_context(tc.tile_pool(name="wdd", bufs=3))
        W1P = ctx.enter_context(tc.tile_pool(name="w1", bufs=3))
        W2P = ctx.enter_context(tc.tile_pool(name="w2", bufs=3))
        HP = ctx.enter_context(tc.tile_pool(name="hbuf", bufs=2))
        BR = ctx.enter_context(tc.tile_pool(name="brow", bufs=2))
        SM = ctx.enter_context(tc.tile_pool(name="small", bufs=4))
        RW = ctx.enter_context(tc.tile_pool(name="rrows", bufs=2))
        CST = ctx.enter_context(tc.tile_pool(name="cst", bufs=1))
        RB = ctx.enter_context(tc.tile_pool(name="rbc", bufs=3))
        PS = ctx.enter_context(tc.tile_pool(name="psum", bufs=6, space="PSUM"))
        PSD = ctx.enter_context(tc.tile_pool(name="psumd", bufs=2, space="PSUM"))

        ident = CST.tile([P, P], f32)
        make_identity(nc, ident)
        eps_t = CST.tile([P, 1], f32)
        nc.vector.memset(eps_t, LN_EPS)
        ones_row = CST.tile([1, P], bf16)
        nc.vector.memset(ones_row, 1.0)

        def arena(shape, dtype):
            return A.tile(shape, dtype, tag="A", name="A")

        def rowbcast(src_1d):
            """DRAM AP [n] -> SBUF [P, n] replicated across partitions."""
            n = src_1d.shape[0]
            t = BR.tile([P, n], f32, tag="brow")
            bc = bass.AP(tensor=src_1d.tensor, offset=src_1d.offset,
                         ap=[[0, P]] + [list(x) for x in src_1d.ap])
            nc.sync.dma_start(t, bc)
            return t

        def transpose_to(dst_T, src_nat):
            """src_nat [P, TT, ncb*128] f32 natural -> dst_T [P, ncb, S] bf16."""
            ncb = dst_T.shape[1]
            for dt in range(ncb):
                for t in range(TT):
                    ps = PS.tile([P, P], f32, tag="ps")
                    nc.tensor.transpose(ps, src_nat[:, t, dt * P:(dt + 1) * P], ident)
                    nc.vector.tensor_copy(dst_T[:, dt, t * P:(t + 1) * P], ps)

        def load_wdd_half(name, l, nh):
            """[P, DT, 512] bf16 tile = W[l][:, nh*512:(nh+1)*512] k-tiled."""
            w = WP.tile([P, DT, 512], bf16, tag="wdd")
            src = Wd[name][l, :, nh * 512:(nh + 1) * 512]
            nc.sync.dma_start(w, src.rearrange("(kt p) n -> p kt n", p=P))
            return w

        def layer(u, l, src1, src2, dst1, dst2):
            # ---- 1) streams + transpose ----
            mT = {}
            for si, src in ((1, src1), (2, src2)):
                m_nat = arena([P, TT, D], f32)
                nc.sync.dma_start(m_nat, src[:, :].rearrange("(t p) d -> p t d", p=P))
                t = arena([P, DT, S], bf16)
                transpose_to(t, m_nat)
                mT[si] = t

            # ---- 2) Q^T, K^T (weights stationary, transposed out) ----
            QT = arena([P, DT, S], bf16)
            KT = arena([P, DT, S], bf16)
            for dstT, wname, bname, m in ((QT, "WQ", "bQ", mT[1]),
                                          (KT, "WK", "bK", mT[2])):
                bcol = SM.tile([P, DT], f32, tag="bcol")
                nc.sync.dma_start(bcol, Wd[bname][l, :].rearrange("(dt p) -> p dt", p=P))
                for nh in range(2):
                    w = load_wdd_half(wname, l, nh)
                    for dh in range(4):
                        dt = nh * 4 + dh
                        ps = PS.tile([P, S], f32, tag="ps")
                        for kt in range(DT):
                            nc.tensor.matmul(ps, w[:, kt, dh * P:(dh + 1) * P],
                                             m[:, kt, :],
                                             start=(kt == 0), stop=(kt == DT - 1))
                        nc.scalar.activation(out=dstT[:, dt, :], in_=ps,
                                             func=AF.Identity,
                                             bias=bcol[:, dt:dt + 1], scale=1.0)

            # ---- 3) V1, V2 (activations stationary, natural out) ----
            V = {}
            for si, wname, bname, m in ((1, "WV1", "bV1", mT[1]),
                                        (2, "WV2", "bV2", mT[2])):
                v = arena([P, TT, D], bf16)
                brow = rowbcast(Wd[bname][l, :])
                for nh in range(2):
                    w = load_wdd_half(wname, l, nh)
                    for t in range(TT):
                        ps = PS.tile([P, S], f32, tag="ps")
                        for kt in range(DT):
                            nc.tensor.matmul(ps, m[:, kt, t * P:(t + 1) * P],
                                             w[:, kt, :],
                                             start=(kt == 0), stop=(kt == DT - 1))
                        sl = slice(nh * 512, (nh + 1) * 512)
                        nc.vector.tensor_add(v[:, t, sl], ps, brow[:, sl])
                V[si] = v

            # ---- 4) attention per head -> U1T, U2T (transposed, normalized) ----
            # U1T[d(P), b] = nV1^T, U2T[d(P), a] = nV2^T
            # Softmax denominators come free from the exp's accum_out (free-dim
            # row sums): EQK accum -> den1[a] (cols), EKQ accum -> den2[b].
            U1T = arena([P, DT, S], bf16)
            U2T = arena([P, DT, S], bf16)
            for h in range(H):
                hp, ho = h // 2, (h % 2) * HD
                EQK = E.tile([P, TT, S], bf16, tag="E")
                EKQ = E.tile([P, TT, S], bf16, tag="E")
                dc1 = SM.tile([P, TT], f32, tag="dc")
                dc2 = SM.tile([P, TT], f32, tag="dc")
                for at in range(TT):
                    ps = PS.tile([P, S], f32, tag="ps")
                    nc.tensor.matmul(ps, QT[ho:ho + HD, hp, at * P:(at + 1) * P],
                                     KT[ho:ho + HD, hp, :], start=True, stop=True)
                    nc.scalar.activation(out=EQK[:, at, :], in_=ps, func=AF.Exp,
                                         scale=SCALE, accum_out=dc1[:, at:at + 1])
                for bt in range(TT):
                    ps = PS.tile([P, S], f32, tag="ps")
                    nc.tensor.matmul(ps, KT[ho:ho + HD, hp, bt * P:(bt + 1) * P],
                                     QT[ho:ho + HD, hp, :], start=True, stop=True)
                    nc.scalar.activation(out=EKQ[:, bt, :], in_=ps, func=AF.Exp,
                                         scale=SCALE, accum_out=dc2[:, bt:bt + 1])
                # reciprocal on tiny column form, transpose to a row, then
                # PE-broadcast the reciprocal row across partitions
                rdc1 = SM.tile([P, TT], f32, tag="dc")
                rdc2 = SM.tile([P, TT], f32, tag="dc")
                nc.vector.reciprocal_approx_fast(rdc1, dc1)
                nc.vector.reciprocal_approx_fast(rdc2, dc2)
                rbc = {}
                for idx, rdc in ((1, rdc1), (2, rdc2)):
                    psd = PSD.tile([1, S], f32, tag="psd")
                    for at in range(TT):
                        nc.tensor.transpose(psd[0:1, at * P:(at + 1) * P],
                                            rdc[:, at:at + 1], ident)
                    rrow = RW.tile([1, S], bf16, tag="rrow")
                    nc.vector.tensor_copy(rrow, psd)
                    psb = PS.tile([P, S], f32, tag="ps")
                    nc.tensor.matmul(psb, ones_row, rrow, start=True, stop=True)
                    rb = RB.tile([P, S], bf16, tag="rb")
                    nc.vector.tensor_copy(rb, psb)
                    rbc[idx] = rb
                # U2T[hd, a] = (sum_b V2[b,hd] EKQ[b,a]) * rden1[a]
                psu = PS.tile([HD, S], f32, tag="ps")
                for bt in range(TT):
                    nc.tensor.matmul(psu, V[2][:, bt, h * HD:(h + 1) * HD],
                                     EKQ[:, bt, :],
                                     start=(bt == 0), stop=(bt == TT - 1))
                nc.vector.tensor_mul(U2T[ho:ho + HD, hp, :], psu, rbc[1][:HD, :])
                # U1T[hd, b] = (sum_a V1[a,hd] EQK[a,b]) * rden2[b]
                psu = PS.tile([HD, S], f32, tag="ps")
                for at in range(TT):
                    nc.tensor.matmul(psu, V[1][:, at, h * HD:(h + 1) * HD],
                                     EQK[:, at, :],
                                     start=(at == 0), stop=(at == TT - 1))
                nc.vector.tensor_mul(U1T[ho:ho + HD, hp, :], psu, rbc[2][:HD, :])

            # ---- 6) O-proj + bias + residual ----
            o = {}
            for si, wname, bname, nvT, src_ in ((1, "WO1", "bO1", U1T, src1),
                                                (2, "WO2", "bO2", U2T, src2)):
                m_res = arena([P, TT, D], f32)
                nc.sync.dma_start(m_res, src_[:, :].rearrange("(t p) d -> p t d", p=P))
                brow = rowbcast(Wd[bname][l, :])
                osi = arena([P, TT, D], f32)
                for nh in range(2):
                    w = load_wdd_half(wname, l, nh)
                    for t in range(TT):
                        ps = PS.tile([P, S], f32, tag="ps")
                        for dm in range(DT):
                            nc.tensor.matmul(ps, nvT[:, dm, t * P:(t + 1) * P],
                                             w[:, dm, :],
                                             start=(dm == 0), stop=(dm == DT - 1))
                        sl = slice(nh * 512, (nh + 1) * 512)
                        nc.vector.tensor_add(osi[:, t, sl], ps, m_res[:, t, sl])
                        nc.vector.tensor_add(osi[:, t, sl], osi[:, t, sl], brow[:, sl])
                o[si] = osi

            # ---- 7) LayerNorm (gamma/beta folded into FFN W1/b1 on host) ----
            x = {}
            for si in (1, 2):
                xsi = arena([P, TT, D], f32)
                for t in range(TT):
                    stats = SM.tile([P, 2, 6], f32, tag="st")
                    for c in range(2):
                        nc.vector.bn_stats(stats[:, c, :], o[si][:, t, c * 512:(c + 1) * 512])
                    mv = SM.tile([P, 2], f32, tag="mv")
                    nc.vector.bn_aggr(mv, stats)
                    rstd = SM.tile([P, 1], f32, tag="rstd")
                    nc.scalar.activation(out=rstd, in_=mv[:, 1:2], func=AF.Sqrt,
                                         bias=eps_t, scale=1.0)
                    nc.vector.reciprocal(rstd, rstd)
                    nc.vector.tensor_scalar(xsi[:, t, :], o[si][:, t, :],
                                            mv[:, 0:1], rstd,
                                            ALU.subtract, ALU.mult)
                x[si] = xsi

            # ---- 8-9) FFN per stream + final residual ----
            out_sb = {}
            for si, w1name, b1name, w2name, b2name, dst in (
                    (1, "F1W1t", "F1b1", "F1W2t", "F1b2", dst1),
                    (2, "F2W1t", "F2b1", "F2W2t", "F2b2", dst2)):
                xT = arena([P, DT, S], bf16)
                transpose_to(xT, x[si])
                b1col = SM.tile([P, HT], f32, tag="b1col")
                nc.sync.dma_start(b1col, Wd[b1name][l, :].rearrange("(ht p) -> p ht", p=P))
                b2row = rowbcast(Wd[b2name][l, :])
                yT = arena([P, DT, S], f32)
                for qr in range(4):
                    hbuf = HP.tile([P, 8, S], bf16, tag="h")
                    for hh in range(8):
                        ht = qr * 8 + hh
                        w1 = W1P.tile([P, DT, P], bf16, tag="w1")
                        nc.sync.dma_start(
                            w1, Wd[w1name][l, ht].rearrange("(kt p) n -> p kt n", p=P))
                        ps = PS.tile([P, S], f32, tag="ps")
                        for kt in range(DT):
                            nc.tensor.matmul(ps, w1[:, kt, :], xT[:, kt, :],
                                             start=(kt == 0), stop=(kt == DT - 1))
                        # relu(h + b1) on the scalar engine
                        nc.scalar.activation(out=hbuf[:, hh, :], in_=ps,
                                             func=AF.Relu,
                                             bias=b1col[:, ht:ht + 1], scale=1.0)
                    # y^T accumulation for this quarter of HID
                    for dm in range(DT):
                        w2 = W2P.tile([P, 8, P], bf16, tag="w2")
                        nc.sync.dma_start(
                            w2, Wd[w2name][l, dm, qr * 1024:(qr + 1) * 1024, :]
                                .rearrange("(ht p) n -> p ht n", p=P))
                        ps = PS.tile([P, S], f32, tag="ps")
                        for hh in range(8):
                            nc.tensor.matmul(ps, w2[:, hh, :], hbuf[:, hh, :],
                                             start=(hh == 0), stop=(hh == 7))
                        if qr == 0:
                            nc.vector.tensor_copy(yT[:, dm, :], ps)
                        else:
                            nc.vector.tensor_add(yT[:, dm, :], yT[:, dm, :], ps)
                # out = o + y + b2 (transpose y back, fused adds), DMA out
                outt = arena([P, TT, D], f32)
                for dm in range(DT):
                    for t in range(TT):
                        ps = PS.tile([P, P], f32, tag="ps")
                        nc.tensor.transpose(ps, yT[:, dm, t * P:(t + 1) * P], ident)
                        sl = slice(dm * P, (dm + 1) * P)
                        nc.vector.tensor_add(outt[:, t, sl], ps, o[si][:, t, sl])
                        nc.vector.tensor_add(outt[:, t, sl], outt[:, t, sl], b2row[:, sl])
                nc.sync.dma_start(dst[:, :].rearrange("(t p) d -> p t d", p=P), outt)
                out_sb[si] = outt
            return out_sb

        # interleave the two units layer-by-layer: each layer's serial tail
        # (y-evac / out-add / transposes) overlaps the other unit's matmuls
        for l in range(NL):
            for u in ("a", "b"):
                src1 = m_in[(u, 1)] if l == 0 else scr[(u, 1)]
                src2 = m_in[(u, 2)] if l == 0 else scr[(u, 2)]
                dst1 = scr[(u, 1)] if l == 0 else outs[(u, 1)]
                dst2 = scr[(u, 2)] if l == 0 else outs[(u, 2)]
                layer(u, l, src1, src2, dst1, dst2)

    nc.compile()
    return nc


# unit u = (branch, batch). core c (0..5): branch c//2, batches (2*(c%2), 2*(c%2)+1)
# cores 6,7 duplicate cores 0,1.
_CORE_UNITS = {c: (c // 2, 2 * (c % 2), 2 * (c % 2) + 1) for c in range(6)}
_CORE_UNITS[6] = _CORE_UNITS[0]
_CORE_UNITS[7] = _CORE_UNITS[1]


def _branch_streams(br, text, audio, visual):
    if br == 0:
        return text, audio
    if br == 1:
        return text, visual
    return audio, visual


def kernel(**inputs):
    import ml_dtypes
    from concourse.bass_utils import run_bass_kernel_spmd

    bf16 = ml_dtypes.bfloat16

    if "nc" not in _cache:
        _cache["nc"] = _build_program()
    nc = _cache["nc"]

    f = lambda k: np.ascontiguousarray(np.asarray(inputs[k], dtype=np.float32))
    text, audio, visual = f("text_features"), f("audio_features"), f("visual_features")

    # per-branch weight bundles (bf16 weights; LN gamma/beta folded into FFN)
    branch_w = []
    for br in range(NBRANCH):
        lsl = slice(br * NL, (br + 1) * NL)
        wb = {}
        for n in ("WQ", "WK", "WV1", "WV2", "WO1", "WO2"):
            wb[n] = f(n)[lsl].astype(bf16)
        for n, src, gname, bname, b1name in (
                ("F1W1t", "F1W1", "LN1g", "LN1b", "F1b1"),
                ("F2W1t", "F2W1", "LN2g", "LN2b", "F2b1")):
            w = f(src)[lsl]                      # [NL, D, HID]
            g = f(gname)[lsl]                    # [NL, D]
            b = f(bname)[lsl]                    # [NL, D]
            wf = g[:, :, None] * w               # fold gamma into W1 rows
            b1f = f(b1name)[lsl] + np.einsum("ld,ldh->lh", b, w)
            wb[n] = np.ascontiguousarray(
                wf.reshape(NL, D, HT, P).transpose(0, 2, 1, 3)).astype(bf16)
            wb[b1name] = b1f.astype(np.float32)
        for n, src in (("F1W2t", "F1W2"), ("F2W2t", "F2W2")):
            w = f(src)[lsl]                      # [NL, HID, D]
            wb[n] = np.ascontiguousarray(
                w.reshape(NL, HID, DT, P).transpose(0, 2, 1, 3)).astype(bf16)
        for dev, host in (("bQ", "bQ"), ("bK", "bK"), ("bV1", "bV1"), ("bV2", "bV2"),
                          ("bO1", "bO1"), ("bO2", "bO2"),
                          ("F1b2", "F1b2"), ("F2b2", "F2b2")):
            wb[dev] = f(host)[lsl]
        branch_w.append(wb)

    in_maps = []
    for c in range(8):
        br, ba, bb = _CORE_UNITS[c]
        s1, s2 = _branch_streams(br, text, audio, visual)
        im = dict(branch_w[br])
        im["m1a"] = np.ascontiguousarray(s1[ba])
        im["m2a"] = np.ascontiguousarray(s2[ba])
        im["m1b"] = np.ascontiguousarray(s1[bb])
        im["m2b"] = np.ascontiguousarray(s2[bb])
        in_maps.append(im)

    res = run_bass_kernel_spmd(nc, in_maps, core_ids=list(range(8)))
    _cache["last_results"] = res

    # assemble outputs: per branch, [B, S, D] for each stream
    out_s1 = [np.zeros((B, S, D), np.float32) for _ in range(NBRANCH)]
    out_s2 = [np.zeros((B, S, D), np.float32) for _ in range(NBRANCH)]
    for c in range(6):
        br, ba, bb = _CORE_UNITS[c]
        r = res.results[c]
        out_s1[br][ba] = r["o1a"]
        out_s2[br][ba] = r["o2a"]
        out_s1[br][bb] = r["o1b"]
        out_s2[br][bb] = r["o2b"]

    return (out_s1[0], out_s2[0], out_s1[1], out_s2[1], out_s1[2], out_s2[2])



# revision 20
# speedup vs baseline: 1.0576x; 1.0576x over previous
"""Trainium2 Bass kernel for nn_DualModalityEnhanced (3-branch dual-stream
cross-attention transformer, B=4 S=512 D=1024 H=16 HID=4096 NL=2).

Sharding: 12 independent (branch, batch) units, each a 2-layer dual-stream
chain. SPMD over 8 cores, 2 units per core (cores 6,7 duplicate work of
cores 0,1; host discards duplicates). All matmuls run as fp32r (TF32-like)
at full PE rate.

Self-contained: hardcodes shapes; builds + compiles the Bass program on
first call and runs it via run_bass_kernel_spmd.
"""

import numpy as np

B, S, D, H, HID, NL = 4, 512, 1024, 16, 4096, 2
HD = D // H            # 64
NBRANCH = 3
LN_EPS = 1e-5
SCALE = 1.0 / 8.0      # 1/sqrt(HD)

P = 128
TT = S // P            # 4 token tiles
DT = D // P            # 8 feature row-tiles
HT = HID // P          # 32 hidden row-tiles
NQ = 4                 # quarter splits of D columns (256 each)

_cache = {}


def _build_program():
    import contextlib
    import concourse.bass as bass
    import concourse.mybir as mybir
    import concourse.tile as tile
    from concourse import bacc
    from concourse.masks import make_identity

    f32 = mybir.dt.float32
    f32r = mybir.dt.float32r
    bf16 = mybir.dt.bfloat16
    AF = mybir.ActivationFunctionType
    ALU = mybir.AluOpType

    nc = bacc.Bacc("TRN2", target_bir_lowering=False, debug=False, num_devices=8)

    # ---------------- external I/O ----------------
    m_in, outs, scr = {}, {}, {}
    for u in ("a", "b"):
        for s in (1, 2):
            m_in[(u, s)] = nc.dram_tensor(f"m{s}{u}", [S, D], f32, kind="ExternalInput")
            outs[(u, s)] = nc.dram_tensor(f"o{s}{u}", [S, D], f32, kind="ExternalOutput")
            scr[(u, s)] = nc.dram_tensor(f"scr{s}{u}", [S, D], f32)

    Wd = {}
    for n in ("WQ", "WK", "WV1", "WV2", "WO1", "WO2"):
        Wd[n] = nc.dram_tensor(n, [NL, D, D], bf16, kind="ExternalInput")
    # host-pretiled FFN weights (bf16, LN gamma folded into W1):
    #   FsW1t[l, ht] = FsW1[l][:, ht*128:(ht+1)*128]   -> [NL, HT, D, P]
    #   FsW2t[l, dm] = FsW2[l][:, dm*128:(dm+1)*128]   -> [NL, DT, HID, P]
    for n in ("F1W1t", "F2W1t"):
        Wd[n] = nc.dram_tensor(n, [NL, HT, D, P], bf16, kind="ExternalInput")
    for n in ("F1W2t", "F2W2t"):
        Wd[n] = nc.dram_tensor(n, [NL, DT, HID, P], bf16, kind="ExternalInput")
    for n in ("bQ", "bK", "bV1", "bV2", "bO1", "bO2", "F1b2", "F2b2"):
        Wd[n] = nc.dram_tensor(n, [NL, D], f32, kind="ExternalInput")
    for n in ("F1b1", "F2b1"):
        Wd[n] = nc.dram_tensor(n, [NL, HID], f32, kind="ExternalInput")

    with tile.TileContext(nc) as tc, contextlib.ExitStack() as ctx:
        A = ctx.enter_context(tc.tile_pool(name="arena", bufs=7))
        E = ctx.enter_context(tc.tile_pool(name="eqk", bufs=5))
        WP = ctx.enter_context(tc.tile_pool(name="wdd", bufs=3))
        W1P = ctx.enter_context(tc.tile_pool(name="w1", bufs=3))
        W2P = ctx.enter_context(tc.tile_pool(name="w2", bufs=3))
        HP = ctx.enter_context(tc.tile_pool(name="hbuf", bufs=2))
        BR = ctx.enter_context(tc.tile_pool(name="brow", bufs=2))
        SM = ctx.enter_context(tc.tile_pool(name="small", bufs=4))
        RW = ctx.enter_context(tc.tile_pool(name="rrows", bufs=2))
        CST = ctx.enter_context(tc.tile_pool(name="cst", bufs=1))
        RB = ctx.enter_context(tc.tile_pool(name="rbc", bufs=3))
        PS = ctx.enter_context(tc.tile_pool(name="psum", bufs=8, space="PSUM"))

        ident = CST.tile([P, P], f32)
        make_identity(nc, ident)
        eps_t = CST.tile([P, 1], f32)
        nc.vector.memset(eps_t, LN_EPS)
        ones_row = CST.tile([1, P], bf16)
        nc.vector.memset(ones_row, 1.0)

        def arena(shape, dtype):
            return A.tile(shape, dtype, tag="A", name="A")

        def rowbcast(src_1d):
            """DRAM AP [n] -> SBUF [P, n] replicated across partitions."""
            n = src_1d.shape[0]
            t = BR.tile([P, n], f32, tag="brow")
            bc = bass.AP(tensor=src_1d.tensor, offset=src_1d.offset,
                         ap=[[0, P]] + [list(x) for x in src_1d.ap])
            nc.sync.dma_start(t, bc)
            return t

        def transpose_to(dst_T, src_nat):
            """src_nat [P, TT, ncb*128] f32 natural -> dst_T [P, ncb, S] bf16."""
            ncb = dst_T.shape[1]
            for dt in range(ncb):
                for t in range(TT):
                    ps = PS.tile([P, P], f32, tag="ps")
                    nc.tensor.transpose(ps, src_nat[:, t, dt * P:(dt + 1) * P], ident)
                    nc.vector.tensor_copy(dst_T[:, dt, t * P:(t + 1) * P], ps)

        def load_wdd_half(name, l, nh):
            """[P, DT, 512] bf16 tile = W[l][:, nh*512:(nh+1)*512] k-tiled."""
            w = WP.tile([P, DT, 512], bf16, tag="wdd")
            src = Wd[name][l, :, nh * 512:(nh + 1) * 512]
            nc.sync.dma_start(w, src.rearrange("(kt p) n -> p kt n", p=P))
            return w

        def layer(u, l, src1, src2, dst1, dst2, m_sb=None):
            # ---- 1) streams (SBUF-resident if given) + transpose ----
            mT = {}
            for si, src in ((1, src1), (2, src2)):
                if m_sb is not None:
                    m_nat = m_sb[si]
                else:
                    m_nat = arena([P, TT, D], f32)
                    nc.sync.dma_start(m_nat, src[:, :].rearrange("(t p) d -> p t d", p=P))
                t = arena([P, DT, S], bf16)
                transpose_to(t, m_nat)
                mT[si] = t

            # ---- 2) Q^T, K^T (weights stationary, transposed out) ----
            QT = arena([P, DT, S], bf16)
            KT = arena([P, DT, S], bf16)
            for dstT, wname, bname, m in ((QT, "WQ", "bQ", mT[1]),
                                          (KT, "WK", "bK", mT[2])):
                bcol = SM.tile([P, DT], f32, tag="bcol")
                nc.sync.dma_start(bcol, Wd[bname][l, :].rearrange("(dt p) -> p dt", p=P))
                for nh in range(2):
                    w = load_wdd_half(wname, l, nh)
                    for dh in range(4):
                        dt = nh * 4 + dh
                        ps = PS.tile([P, S], f32, tag="ps")
                        for kt in range(DT):
                            nc.tensor.matmul(ps, w[:, kt, dh * P:(dh + 1) * P],
                                             m[:, kt, :],
                                             start=(kt == 0), stop=(kt == DT - 1))
                        nc.scalar.activation(out=dstT[:, dt, :], in_=ps,
                                             func=AF.Identity,
                                             bias=bcol[:, dt:dt + 1], scale=1.0)

            # ---- 3) V1, V2 (activations stationary, natural out) ----
            V = {}
            for si, wname, bname, m in ((1, "WV1", "bV1", mT[1]),
                                        (2, "WV2", "bV2", mT[2])):
                v = arena([P, TT, D], bf16)
                brow = rowbcast(Wd[bname][l, :])
                for nh in range(2):
                    w = load_wdd_half(wname, l, nh)
                    for t in range(TT):
                        ps = PS.tile([P, S], f32, tag="ps")
                        for kt in range(DT):
                            nc.tensor.matmul(ps, m[:, kt, t * P:(t + 1) * P],
                                             w[:, kt, :],
                                             start=(kt == 0), stop=(kt == DT - 1))
                        sl = slice(nh * 512, (nh + 1) * 512)
                        nc.vector.tensor_add(v[:, t, sl], ps, brow[:, sl])
                V[si] = v

            # ---- 4) attention per head -> U1T, U2T (transposed, normalized) ----
            # U1T[d(P), b] = nV1^T, U2T[d(P), a] = nV2^T
            # Softmax denominators come free from the exp's accum_out (free-dim
            # row sums): EQK accum -> den1[a] (cols), EKQ accum -> den2[b].
            U1T = arena([P, DT, S], bf16)
            U2T = arena([P, DT, S], bf16)
            for h in range(H):
                hp, ho = h // 2, (h % 2) * HD
                EQK = E.tile([P, TT, S], bf16, tag="E")
                EKQ = E.tile([P, TT, S], bf16, tag="E")
                dc1 = SM.tile([P, TT], f32, tag="dc")
                dc2 = SM.tile([P, TT], f32, tag="dc")
                for at in range(TT):
                    ps = PS.tile([P, S], f32, tag="ps")
                    nc.tensor.matmul(ps, QT[ho:ho + HD, hp, at * P:(at + 1) * P],
                                     KT[ho:ho + HD, hp, :], start=True, stop=True)
                    nc.scalar.activation(out=EQK[:, at, :], in_=ps, func=AF.Exp,
                                         scale=SCALE, accum_out=dc1[:, at:at + 1])
                for bt in range(TT):
                    ps = PS.tile([P, S], f32, tag="ps")
                    nc.tensor.matmul(ps, KT[ho:ho + HD, hp, bt * P:(bt + 1) * P],
                                     QT[ho:ho + HD, hp, :], start=True, stop=True)
                    nc.scalar.activation(out=EKQ[:, bt, :], in_=ps, func=AF.Exp,
                                         scale=SCALE, accum_out=dc2[:, bt:bt + 1])
                # reciprocal on tiny column form, transpose to a row, then
                # PE-broadcast the reciprocal row across partitions
                rdc1 = SM.tile([P, TT], f32, tag="dc")
                rdc2 = SM.tile([P, TT], f32, tag="dc")
                nc.vector.reciprocal_approx_fast(rdc1, dc1)
                nc.vector.reciprocal_approx_fast(rdc2, dc2)
                # each rden broadcast is consumed immediately by its U multiply:
                #   U2T[hd, a] = (sum_b V2[b,hd] EKQ[b,a]) * rden1[a]
                #   U1T[hd, b] = (sum_a V1[a,hd] EQK[a,b]) * rden2[b]
                for rdc, Vt, Et, dstU in ((rdc1, V[2], EKQ, U2T),
                                          (rdc2, V[1], EQK, U1T)):
                    psd = PS.tile([1, S], f32, tag="ps")
                    for at in range(TT):
                        nc.tensor.transpose(psd[0:1, at * P:(at + 1) * P],
                                            rdc[:, at:at + 1], ident)
                    rrow = RW.tile([1, S], bf16, tag="rrow")
                    nc.vector.tensor_copy(rrow, psd)
                    psb = PS.tile([P, S], f32, tag="ps")
                    nc.tensor.matmul(psb, ones_row, rrow, start=True, stop=True)
                    rb = RB.tile([P, S], bf16, tag="rb")
                    nc.vector.tensor_copy(rb, psb)
                    psu = PS.tile([HD, S], f32, tag="ps")
                    for bt in range(TT):
                        nc.tensor.matmul(psu, Vt[:, bt, h * HD:(h + 1) * HD],
                                         Et[:, bt, :],
                                         start=(bt == 0), stop=(bt == TT - 1))
                    nc.vector.tensor_mul(dstU[ho:ho + HD, hp, :], psu, rb[:HD, :])

            # ---- 6) O-proj + bias + residual ----
            o = {}
            for si, wname, bname, nvT, src_ in ((1, "WO1", "bO1", U1T, src1),
                                                (2, "WO2", "bO2", U2T, src2)):
                m_res = arena([P, TT, D], f32)
                nc.sync.dma_start(m_res, src_[:, :].rearrange("(t p) d -> p t d", p=P))
                brow = rowbcast(Wd[bname][l, :])
                osi = arena([P, TT, D], f32)
                for nh in range(2):
                    w = load_wdd_half(wname, l, nh)
                    for t in range(TT):
                        ps = PS.tile([P, S], f32, tag="ps")
                        for dm in range(DT):
                            nc.tensor.matmul(ps, nvT[:, dm, t * P:(t + 1) * P],
                                             w[:, dm, :],
                                             start=(dm == 0), stop=(dm == DT - 1))
                        sl = slice(nh * 512, (nh + 1) * 512)
                        nc.vector.tensor_add(osi[:, t, sl], ps, m_res[:, t, sl])
                        nc.vector.tensor_add(osi[:, t, sl], osi[:, t, sl], brow[:, sl])
                o[si] = osi

            # ---- 7) LayerNorm (gamma/beta folded into FFN W1/b1 on host) ----
            x = {}
            for si in (1, 2):
                xsi = arena([P, TT, D], f32)
                for t in range(TT):
                    stats = SM.tile([P, 2, 6], f32, tag="st")
                    for c in range(2):
                        nc.vector.bn_stats(stats[:, c, :], o[si][:, t, c * 512:(c + 1) * 512])
                    mv = SM.tile([P, 2], f32, tag="mv")
                    nc.vector.bn_aggr(mv, stats)
                    rstd = SM.tile([P, 1], f32, tag="rstd")
                    nc.scalar.activation(out=rstd, in_=mv[:, 1:2], func=AF.Sqrt,
                                         bias=eps_t, scale=1.0)
                    nc.vector.reciprocal(rstd, rstd)
                    nc.vector.tensor_scalar(xsi[:, t, :], o[si][:, t, :],
                                            mv[:, 0:1], rstd,
                                            ALU.subtract, ALU.mult)
                x[si] = xsi

            # ---- 8-9) FFN per stream + final residual ----
            out_sb = {}
            for si, w1name, b1name, w2name, b2name, dst in (
                    (1, "F1W1t", "F1b1", "F1W2t", "F1b2", dst1),
                    (2, "F2W1t", "F2b1", "F2W2t", "F2b2", dst2)):
                xT = arena([P, DT, S], bf16)
                transpose_to(xT, x[si])
                b1col = SM.tile([P, HT], f32, tag="b1col")
                nc.sync.dma_start(b1col, Wd[b1name][l, :].rearrange("(ht p) -> p ht", p=P))
                b2row = rowbcast(Wd[b2name][l, :])
                yT = arena([P, DT, S], f32)
                for qr in range(4):
                    hbuf = HP.tile([P, 8, S], bf16, tag="h")
                    for hh in range(8):
                        ht = qr * 8 + hh
                        w1 = W1P.tile([P, DT, P], bf16, tag="w1")
                        nc.sync.dma_start(
                            w1, Wd[w1name][l, ht].rearrange("(kt p) n -> p kt n", p=P))
                        ps = PS.tile([P, S], f32, tag="ps")
                        for kt in range(DT):
                            nc.tensor.matmul(ps, w1[:, kt, :], xT[:, kt, :],
                                             start=(kt == 0), stop=(kt == DT - 1))
                        # relu(h + b1) on the scalar engine
                        nc.scalar.activation(out=hbuf[:, hh, :], in_=ps,
                                             func=AF.Relu,
                                             bias=b1col[:, ht:ht + 1], scale=1.0)
                    # y^T accumulation for this quarter of HID
                    for dm in range(DT):
                        w2 = W2P.tile([P, 8, P], bf16, tag="w2")
                        nc.sync.dma_start(
                            w2, Wd[w2name][l, dm, qr * 1024:(qr + 1) * 1024, :]
                                .rearrange("(ht p) n -> p ht n", p=P))
                        ps = PS.tile([P, S], f32, tag="ps")
                        for hh in range(8):
                            nc.tensor.matmul(ps, w2[:, hh, :], hbuf[:, hh, :],
                                             start=(hh == 0), stop=(hh == 7))
                        if qr == 0:
                            nc.vector.tensor_copy(yT[:, dm, :], ps)
                        else:
                            nc.vector.tensor_add(yT[:, dm, :], yT[:, dm, :], ps)
                # out = o + y + b2 (transpose y back, fused adds), DMA out
                outt = arena([P, TT, D], f32)
                for dm in range(DT):
                    for t in range(TT):
                        ps = PS.tile([P, P], f32, tag="ps")
                        nc.tensor.transpose(ps, yT[:, dm, t * P:(t + 1) * P], ident)
                        sl = slice(dm * P, (dm + 1) * P)
                        nc.vector.tensor_add(outt[:, t, sl], ps, o[si][:, t, sl])
                        nc.vector.tensor_add(outt[:, t, sl], outt[:, t, sl], b2row[:, sl])
                nc.sync.dma_start(dst[:, :].rearrange("(t p) d -> p t d", p=P), outt)
                out_sb[si] = outt
            return out_sb

        for u in ("a", "b"):
            m_sb = None
            for l in range(NL):
                src1 = m_in[(u, 1)] if l == 0 else scr[(u, 1)]
                src2 = m_in[(u, 2)] if l == 0 else scr[(u, 2)]
                dst1 = scr[(u, 1)] if l == 0 else outs[(u, 1)]
                dst2 = scr[(u, 2)] if l == 0 else outs[(u, 2)]
                m_sb = layer(u, l, src1, src2, dst1, dst2, m_sb=m_sb)

    nc.compile()
    return nc


# unit u = (branch, batch). core c (0..5): branch c//2, batches (2*(c%2), 2*(c%2)+1)
# cores 6,7 duplicate cores 0,1.
_CORE_UNITS = {c: (c // 2, 2 * (c % 2), 2 * (c % 2) + 1) for c in range(6)}
_CORE_UNITS[6] = _CORE_UNITS[0]
_CORE_UNITS[7] = _CORE_UNITS[1]


def _branch_streams(br, text, audio, visual):
    if br == 0:
        return text, audio
    if br == 1:
        return text, visual
    return audio, visual


def kernel(**inputs):
    import ml_dtypes
    from concourse.bass_utils import run_bass_kernel_spmd

    bf16 = ml_dtypes.bfloat16

    if "nc" not in _cache:
        _cache["nc"] = _build_program()
    nc = _cache["nc"]

    f = lambda k: np.ascontiguousarray(np.asarray(inputs[k], dtype=np.float32))
    text, audio, visual = f("text_features"), f("audio_features"), f("visual_features")

    # per-branch weight bundles (bf16 weights; LN gamma/beta folded into FFN)
    branch_w = []
    for br in range(NBRANCH):
        lsl = slice(br * NL, (br + 1) * NL)
        wb = {}
        for n in ("WQ", "WK", "WV1", "WV2", "WO1", "WO2"):
            wb[n] = f(n)[lsl].astype(bf16)
        for n, src, gname, bname, b1name in (
                ("F1W1t", "F1W1", "LN1g", "LN1b", "F1b1"),
                ("F2W1t", "F2W1", "LN2g", "LN2b", "F2b1")):
            w = f(src)[lsl]                      # [NL, D, HID]
            g = f(gname)[lsl]                    # [NL, D]
            b = f(bname)[lsl]                    # [NL, D]
            wf = g[:, :, None] * w               # fold gamma into W1 rows
            b1f = f(b1name)[lsl] + np.einsum("ld,ldh->lh", b, w)
            wb[n] = np.ascontiguousarray(
                wf.reshape(NL, D, HT, P).transpose(0, 2, 1, 3)).astype(bf16)
            wb[b1name] = b1f.astype(np.float32)
        for n, src in (("F1W2t", "F1W2"), ("F2W2t", "F2W2")):
            w = f(src)[lsl]                      # [NL, HID, D]
            wb[n] = np.ascontiguousarray(
                w.reshape(NL, HID, DT, P).transpose(0, 2, 1, 3)).astype(bf16)
        for dev, host in (("bQ", "bQ"), ("bK", "bK"), ("bV1", "bV1"), ("bV2", "bV2"),
                          ("bO1", "bO1"), ("bO2", "bO2"),
                          ("F1b2", "F1b2"), ("F2b2", "F2b2")):
            wb[dev] = f(host)[lsl]
        branch_w.append(wb)

    in_maps = []
    for c in range(8):
        br, ba, bb = _CORE_UNITS[c]
        s1, s2 = _branch_streams(br, text, audio, visual)
        im = dict(branch_w[br])
        im["m1a"] = np.ascontiguousarray(s1[ba])
        im["m2a"] = np.ascontiguousarray(s2[ba])
        im["m1b"] = np.ascontiguousarray(s1[bb])
        im["m2b"] = np.ascontiguousarray(s2[bb])
        in_maps.append(im)

    res = run_bass_kernel_spmd(nc, in_maps, core_ids=list(range(8)))
    _cache["last_results"] = res

    # assemble outputs: per branch, [B, S, D] for each stream
    out_s1 = [np.zeros((B, S, D), np.float32) for _ in range(NBRANCH)]
    out_s2 = [np.zeros((B, S, D), np.float32) for _ in range(NBRANCH)]
    for c in range(6):
        br, ba, bb = _CORE_UNITS[c]
        r = res.results[c]
        out_s1[br][ba] = r["o1a"]
        out_s2[br][ba] = r["o2a"]
        out_s1[br][bb] = r["o1b"]
        out_s2[br][bb] = r["o2b"]

    return (out_s1[0], out_s2[0], out_s1[1], out_s2[1], out_s1[2], out_s2[2])

